# revision 26
# baseline (speedup 1.0000x reference)
"""Causal multi-head attention (b=2, n=2048, dim=1024, 16 heads) on 8 trn2
NeuronCores.

Sharding: core j = 4*g + r owns batch g and heads 4r..4r+3 (tensor parallel
over heads within each batch's 4-core group). Each core:
  P1  projects q/k (transposed layout [head_dim, tokens]) and v (natural
      [tokens, head_dim], ones-augmented) for its 4 heads from x^T, in bf16.
      xt is tiled by token-quarter so v/qk groups start as quarters land.
  P2  causal attention per head pair in S^T orientation: exp without max
      subtraction (scores are O(1)), triangular mask on diagonal tiles,
      O'^T = V_aug.T @ expS^T accumulated in PSUM (row 64 = softmax
      denominator Z), normalization via fast-approx reciprocal + DRAM-bounce
      partition broadcast.
  A2A one 4-core group AllToAll per head pair ([[0..3],[4..7]] replica
      groups, bf16, 512KB): transposes A^T from head-sharded to
      token-sharded. The first overlaps the second pair's attention.
  P3  out = A^T.T @ Wout for this core's 512-token block: the first
      contraction half runs during the second AllToAll, the rest after.
Host: transposes x per batch, slices weights per head group (bf16), gathers
the 8 [512, 1024] row blocks into the full [2, 2048, 1024] output.
"""
import numpy as np
import ml_dtypes

import concourse.bass as bass
import concourse.mybir as mybir
import concourse.tile as tile
from concourse.bass import AP, ds
from concourse.bass_utils import run_bass_kernel_spmd
from concourse.vector_clock import ScopedClock

F32 = mybir.dt.float32
BF16 = mybir.dt.bfloat16
EXP = mybir.ActivationFunctionType.Exp

N_CORES = 8
B, N, DIM, H = 2, 2048, 1024, 16
D = DIM // H                 # 64
HL = 4                       # heads per core
KT = DIM // 128              # 8 contraction k-tiles
NQ = 4                       # token quarters (512 tokens each)
NJ = N // 128                # 16 key tiles per batch
NI = N // 512                # 4 query i-blocks per batch
SCALE = float(D) ** -0.5
VS = 128                     # per-head stride in v tiles (bf16, 256B aligned)

MD = BF16


def _split_multi_waits(nc):
    """This walrus build rejects instructions carrying more than one sync
    wait. Hoist extra waits onto same-engine NoOps inserted directly before
    the offending instruction (engines execute their stream in order, so
    this preserves semantics)."""
    n = 0
    for f in nc.m.functions:
        for bb in f.blocks:
            insts = bb.instructions
            out = []
            changed = False
            for inst in insts:
                si = inst.sync_info
                waits = list(si.on_wait) if si is not None and si.on_wait else []
                if len(waits) > 1:
                    changed = True
                    for w in waits[:-1]:
                        nop = mybir.InstNoOp(name=f"I-waitfix-{n}", ins=[],
                                             outs=[])
                        n += 1
                        nop.engine = inst.engine
                        nop.sync_info = mybir.SyncInfo(on_wait=[w],
                                                       on_update=[])
                        out.append(nop)
                    si.on_wait = waits[-1:]
                out.append(inst)
            if changed:
                insts[:] = out
    return n


class _TC(tile.TileContext):
    """Tail drain in this walrus build only supports one sync-wait per CTRL
    instruction; spread the residual global-clock waits over SP nops, and
    split any remaining multi-wait instructions after scheduling."""

    def _drain_and_barrier(self, tick_clock, wait_clock):
        nop = self.nc.sync.nop()
        wait_clock.add_sem_waits(nop.ins, ScopedClock({None: tick_clock.global_clock}))
        si = nop.ins.sync_info
        waits = list(si.on_wait or []) if si is not None else []
        if len(waits) > 1:
            si.on_wait = waits[:1]
            for w in waits[1:]:
                extra = self.nc.sync.nop()
                extra.ins.sync_info = mybir.SyncInfo(on_wait=[w], on_update=[])
        self.nc.sync.drain()
        self.nc.all_engine_barrier()
        assert self.sems is not None
        popped = self.nc._tile_sem_poison_stack.pop()
        assert popped is self._sem_poison
        self.nc.clear_and_free_semaphores(list(self.sems.allocated().values()))
        self.nc.all_engine_barrier()

    def __exit__(self, exc_type, exc_val, exc_tb):
        r = super().__exit__(exc_type, exc_val, exc_tb)
        if exc_type is None:
            _split_multi_waits(self.nc)
        return r


def _bcast(src_dram_row, parts):
    """DRAM [1, n] row -> AP replicating it over `parts` partitions (step-0
    leading dim; only legal for DRAM sources)."""
    return AP(src_dram_row.tensor, src_dram_row.offset,
              [[0, parts]] + list(src_dram_row.ap)[1:])


def _build():
    nc = bass.Bass(trn_type="TRN2", target_bir_lowering=False, debug=False,
                   num_devices=N_CORES)
    dt = F32
    # xt pre-tiled on host: [128, (quarter, ktile, 512)] so each quarter is
    # one fully-linear 8KB-per-partition DMA and unlocks that quarter's
    # v/q/k projection groups.
    xt_d = nc.dram_tensor("xt", [128, NQ * KT * 512], MD, kind="ExternalInput").ap()
    wq_d = nc.dram_tensor("wq", [128, KT * HL * D], MD, kind="ExternalInput").ap()
    wk_d = nc.dram_tensor("wk", [128, KT * HL * D], MD, kind="ExternalInput").ap()
    wv_d = nc.dram_tensor("wv", [128, KT * HL * D], MD, kind="ExternalInput").ap()
    wout_d = nc.dram_tensor("wout", [128, KT * DIM], MD, kind="ExternalInput").ap()
    bq_d = nc.dram_tensor("bq", [HL * D, 1], dt, kind="ExternalInput").ap()
    bk_d = nc.dram_tensor("bk", [HL * D, 1], dt, kind="ExternalInput").ap()
    bv_d = nc.dram_tensor("bv", [1, HL * D], dt, kind="ExternalInput").ap()
    bout_d = nc.dram_tensor("bout", [1, DIM], dt, kind="ExternalInput").ap()
    mask_d = nc.dram_tensor("mask", [128, 128], MD, kind="ExternalInput").ap()
    ident_d = nc.dram_tensor("ident", [128, 128], MD, kind="ExternalInput").ap()
    ones_d = nc.dram_tensor("ones", [1, HL], MD, kind="ExternalInput").ap()
    out_d = nc.dram_tensor("out", [N // HL, DIM], dt, kind="ExternalOutput").ap()

    with _TC(nc) as tc, \
            nc.allow_low_precision(reason="bf16 matmul operand staging"):
        _body(nc, tc, xt_d, wq_d, wk_d, wv_d, wout_d, bq_d, bk_d, bv_d,
              bout_d, mask_d, ident_d, ones_d, out_d)
    return nc


def _body(nc, tc, xt_d, wq_d, wk_d, wv_d, wout_d, bq_d, bk_d, bv_d, bout_d,
          mask_d, ident_d, ones_d, out_d):
    mm = nc.tensor.matmul
    with tc.tile_pool(name="persist", bufs=1) as pers:
        # Persistent SBUF: q^T/k^T per head pair, v (ones-augmented) per
        # 128-token tile, mask, biases.
        qt = [pers.tile([128, N], MD, tag=f"qt{p}", name=f"qt{p}") for p in (0, 1)]
        kt = [pers.tile([128, N], MD, tag=f"kt{p}", name=f"kt{p}") for p in (0, 1)]
        vt = [pers.tile([128, HL * VS], MD, tag=f"v{t}", name=f"v{t}")
              for t in range(NJ)]
        mask_sb = pers.tile([128, 128], MD, tag="mask", name="mask_sb")
        ident_sb = pers.tile([128, 128], MD, tag="ident", name="ident_sb")
        bqc = pers.tile([128, 2], F32, tag="bqc", name="bqc")
        bkc = pers.tile([128, 2], F32, tag="bkc", name="bkc")
        bvb = pers.tile([128, HL * D], F32, tag="bvb", name="bvb")
        boutb = pers.tile([128, DIM], F32, tag="boutb", name="boutb")
        ones_sb = pers.tile([128, HL], MD, tag="ones", name="ones_sb")

        # ---------------- P1: projections ----------------
        with (tc.tile_pool(name="p1s", bufs=1) as p1s,
              tc.tile_pool(name="pqk", bufs=4, space="PSUM") as pqk,
              tc.tile_pool(name="pv", bufs=3, space="PSUM") as pvp):
            w_sb = {}
            for nm in ("wq", "wk", "wv"):
                w_sb[nm] = p1s.tile([128, KT, HL * D], MD, tag=nm, name=f"{nm}_sb")
            xt_sb = p1s.tile([128, NQ, KT, 512], MD, tag="xt", name="xt_sb")
            gate = p1s.tile([1, 4], F32, tag="gate", name="gate")
            xt_dv = xt_d.rearrange("p (q k n) -> p q k n", q=NQ, k=KT)
            # first-needed tensors ride the empty HWDGE queues; the rest go
            # via gpsimd SWDGE. Quarters 1-3 are gated behind a consumer of
            # the previous quarter (the SDMA engines round-robin all queued
            # work, so ungated issue would starve quarter 0).
            nc.sync.dma_start(w_sb["wq"][:],
                              wq_d.rearrange("p (k e) -> p k e", k=KT))
            nc.scalar.dma_start(w_sb["wk"][:],
                                wk_d.rearrange("p (k e) -> p k e", k=KT))
            nc.gpsimd.dma_start(w_sb["wv"][:],
                                wv_d.rearrange("p (k e) -> p k e", k=KT))
            nc.sync.dma_start(xt_sb[:, 0, :, :], xt_dv[:, 0, :, :])
            nc.gpsimd.dma_start(mask_sb[:], mask_d[:])
            nc.gpsimd.dma_start(ident_sb[:], ident_d[:])
            nc.gpsimd.dma_start(bqc[:],
                                bq_d.rearrange("(m p) o -> p (m o)", p=128))
            nc.gpsimd.dma_start(bkc[:],
                                bk_d.rearrange("(m p) o -> p (m o)", p=128))
            nc.gpsimd.dma_start(bvb[:], _bcast(bv_d[0:1, :], 128))
            nc.gpsimd.dma_start(boutb[:], _bcast(bout_d[0:1, :], 128))
            nc.gpsimd.dma_start(ones_sb[:], _bcast(ones_d[0:1, :], 128))
            # warm the exp table during the input DMA wait (first real exp
            # otherwise pays the ~2.7us ACT table load inside P2)
            nc.scalar.activation(gate[0:1, 0:1], bqc[0:1, 0:1], EXP, scale=0.0)

            for ntq in range(NQ):
                if ntq > 0:
                    # gate quarter ntq's load behind first use of the
                    # previous quarter (vt tile of its first token tile)
                    nc.gpsimd.tensor_add(gate[0:1, ntq:ntq + 1],
                                         xt_sb[0:1, ntq, 0, 0:1],
                                         vt[4 * (ntq - 1)][0:1, 0:1])
                    eng = (nc.scalar, nc.sync, nc.gpsimd)[ntq - 1]
                    eng.dma_start(xt_sb[:, ntq, :, :], xt_dv[:, ntq, :, :])
                # v projections for this quarter's 4 token tiles
                for tl in range(4):
                    tt = 4 * ntq + tl
                    ps = pvp.tile([128, HL * D], F32, tag="pv", name="ps_v")
                    for kk in range(KT):
                        mm(ps[:],
                           xt_sb[:, ntq, kk, 128 * tl:128 * tl + 128],
                           w_sb["wv"][:, kk, :],
                           start=(kk == 0), stop=(kk == KT - 1))
                    vv = vt[tt].rearrange("p (h x) -> p h x", x=VS)
                    nc.vector.tensor_add(vv[:, :, 0:D],
                                         ps.rearrange("p (h x) -> p h x", x=D),
                                         bvb.rearrange("p (h x) -> p h x", x=D))
                    nc.vector.tensor_copy(
                        vv[:, :, D:D + 1],
                        ones_sb.rearrange("p (h o) -> p h o", o=1))
                # q/k projections for this quarter's 512 tokens
                for w, bcol, dst in (("wq", bqc, qt), ("wk", bkc, kt)):
                    for mt in (0, 1):
                        ps = pqk.tile([128, 512], F32, tag="pqk", name="ps_qk")
                        for kk in range(KT):
                            mm(ps[:],
                               w_sb[w][:, kk, 128 * mt:128 * mt + 128],
                               xt_sb[:, ntq, kk, :],
                               start=(kk == 0), stop=(kk == KT - 1))
                        nc.vector.tensor_scalar_add(
                            dst[mt][:, 512 * ntq:512 * ntq + 512], ps[:],
                            bcol[:, mt:mt + 1])

        # wout prefetch: gated behind the end of P1 so it does not steal
        # HBM bandwidth from the xt quarters; it only needs to land before
        # P3-h0 (~60us later).
        with tc.tile_pool(name="p3w", bufs=1) as p3w:
            wout_sb = p3w.tile([128, KT, DIM], MD, tag="wout", name="wout_sb")
            wgate = p3w.tile([1, 1], F32, tag="wgate", name="wgate")
            nc.gpsimd.tensor_add(wgate[0:1, 0:1], wout_sb[0:1, 0, 0:1],
                                 qt[1][0:1, N - 1:N])
            nc.gpsimd.dma_start(wout_sb[:],
                                wout_d.rearrange("p (k c) -> p k c", k=KT))
            atf = [p3w.tile([128, 4, 512], MD, tag=f"atf{h}", name=f"atf{h}")
                   for h in (0, 1)]

            # ---------------- P2: attention + streamed a2a ----------------
            pid = nc.sync.partition_id()
            gsel = nc.sync.snap(pid // 4, min_val=0, max_val=1)
            with (tc.tile_pool(name="p2s", bufs=4) as p2s,
                  tc.tile_pool(name="p2n", bufs=2) as p2n,
                  tc.tile_pool(name="dram", bufs=1, space="DRAM") as dram,
                  tc.tile_pool(name="p2d", bufs=2, space="DRAM") as p2d,
                  tc.tile_pool(name="sp", bufs=2, space="PSUM") as sp,
                  tc.tile_pool(name="op", bufs=2, space="PSUM") as op):
                # one 8-core AllToAll per head pair (bf16 halves the wire
                # bytes) so the first overlaps the second pair's attention.
                # Every core writes its chunks at BOTH groups' positions
                # (static offsets); receivers pick their group's half with
                # one dynamic read.
                a2a_in = [dram.tile([1024, 512], MD, name=f"a2a_in{h}")
                          for h in (0, 1)]
                a2a_out = [dram.tile([1024, 512], MD, name=f"a2a_out{h}")
                           for h in (0, 1)]
                for pp in (0, 1):
                    for I in range(NI):
                        i0 = 512 * I
                        last = 4 * I + 3
                        poA = op.tile([D + 1, 512], F32, tag="oA", name="poA")
                        poB = op.tile([D + 1, 512], F32, tag="oB", name="poB")
                        for jj in range(4 * I + 4):
                            di = jj - 4 * I
                            f0 = 128 * di if di >= 0 else 0
                            diag = di >= 0
                            ps = sp.tile([128, 1024], F32, tag="s", name="ps_s")
                            mm(ps[:, f0:512],
                               kt[pp][0:64, 128 * jj:128 * jj + 128],
                               qt[pp][0:64, i0 + f0:i0 + 512],
                               start=True, stop=not diag)
                            mm(ps[:, 512 + f0:1024],
                               kt[pp][64:128, 128 * jj:128 * jj + 128],
                               qt[pp][64:128, i0 + f0:i0 + 512],
                               start=True, stop=not diag)
                            if diag:
                                # additive causal mask accumulated on the PE
                                # (I.T @ maskneg = maskneg): keeps the mask
                                # off the DVE/gpsimd streams entirely
                                mm(ps[:, f0:f0 + 128], ident_sb[:],
                                   mask_sb[:], start=False, stop=True,
                                   skip_group_check=True)
                                mm(ps[:, 512 + f0:512 + f0 + 128],
                                   ident_sb[:], mask_sb[:],
                                   start=False, stop=True,
                                   skip_group_check=True)
                            e = p2s.tile([128, 1024], MD, tag="e", name="e_s")
                            ev = e.rearrange("p (h x) -> p h x", x=512)
                            pv2 = ps.rearrange("p (h x) -> p h x", x=512)
                            nc.scalar.activation(ev[:, :, f0:512],
                                                 pv2[:, :, f0:512], EXP,
                                                 scale=SCALE)
                            vv = vt[jj].rearrange("p (h x) -> p h x", x=VS)
                            mm(poA[:, f0:512], vv[:, 2 * pp, 0:D + 1],
                               e[:, f0:512],
                               start=(jj == 0), stop=(jj == last))
                            mm(poB[:, f0:512], vv[:, 2 * pp + 1, 0:D + 1],
                               e[:, 512 + f0:1024],
                               start=(jj == 0), stop=(jj == last))
                        # normalization: Z sits in row 64 of each O' psum.
                        # 1/Z (fast approx) on partition 64, bounced via DRAM
                        # to broadcast across partitions (step-0 DMA).
                        tail_blk = pp == 1 and I == NI - 1
                        oc = p2n.tile([128, 1024], F32, tag="oc", name="oc")
                        nc.vector.tensor_copy(oc[0:65, 0:512], poA[0:65, :])
                        nc.vector.tensor_copy(oc[0:65, 512:1024], poB[0:65, :])
                        zrow = p2n.tile([1, 1024], F32, tag="zrow", name="zrow")
                        zdram = p2d.tile([1, 1024], F32, tag="zdram", name="zdram")
                        if tail_blk:
                            # the ACT engine is idle after the last exp and
                            # 1/Z = exp(-ln Z) there is ~3x faster than the
                            # DVE reciprocal that would otherwise sit on the
                            # critical tail chain
                            lnz = p2n.tile([1, 1024], F32, tag="lnz",
                                           name="lnz")
                            nc.scalar.activation(
                                lnz[0:1, :], oc[64:65, :],
                                mybir.ActivationFunctionType.Ln)
                            nc.scalar.activation(zrow[0:1, :], lnz[0:1, :],
                                                 EXP, scale=-1.0)
                            nc.sync.dma_start(zdram[0:1, :], zrow[0:1, :])
                        else:
                            # split per half so the DRAM bounce of half A
                            # overlaps the reciprocal of half B
                            nc.vector.reciprocal(zrow[0:1, 0:512],
                                                 oc[64:65, 0:512])
                            nc.sync.dma_start(zdram[0:1, 0:512],
                                              zrow[0:1, 0:512])
                            nc.vector.reciprocal(zrow[0:1, 512:1024],
                                                 oc[64:65, 512:1024])
                            nc.sync.dma_start(zdram[0:1, 512:1024],
                                              zrow[0:1, 512:1024])
                        rzb = p2n.tile([64, 1024], F32, tag="rzb", name="rzb")
                        nc.sync.dma_start(rzb[:, 0:512],
                                          _bcast(zdram[0:1, 0:512], 64))
                        nc.sync.dma_start(rzb[:, 512:1024],
                                          _bcast(zdram[0:1, 512:1024], 64))
                        stA = p2n.tile([64, 512], MD, tag="stA", name="stA")
                        stB = p2n.tile([64, 512], MD, tag="stB", name="stB")
                        nc.vector.tensor_mul(stA[:], oc[0:64, 0:512],
                                             rzb[:, 0:512])
                        nc.vector.tensor_mul(stB[:], oc[0:64, 512:1024],
                                             rzb[:, 512:1024])
                        for gg in (0, 1):
                            r0 = 512 * gg + 128 * I
                            nc.sync.dma_start(a2a_in[pp][r0:r0 + 64, :], stA[:])
                            nc.sync.dma_start(a2a_in[pp][r0 + 64:r0 + 128, :],
                                              stB[:])
                    nc.gpsimd.collective_compute(
                        "AllToAll", mybir.AluOpType.bypass,
                        replica_groups=[list(range(N_CORES))],
                        ins=[a2a_in[pp].opt()], outs=[a2a_out[pp].opt()])
                    av = a2a_out[pp].rearrange("(G s p) c -> p G s c",
                                               s=4, p=128)
                    for s in range(4):
                        nc.sync.dma_start(atf[pp][:, s, :],
                                          av[:, ds(gsel, 1), s, :])

            # ---------------- P3: output projection ----------------
            # first contraction half (head pair 0) runs while the second
            # AllToAll is still in flight; second half lands on top.
            # Both ct halves of a (it, k4) pair share the stationary operand.
            with (tc.tile_pool(name="p3s", bufs=2) as p3s,
                  tc.tile_pool(name="p3p", bufs=8, space="PSUM") as p3p):
                pouts = {}
                for h in (0, 1):
                    for it in range(4):
                        for ct in range(2):
                            if h == 0:
                                pouts[(it, ct)] = p3p.tile(
                                    [128, 512], F32, tag=f"po{it}{ct}",
                                    name=f"po{it}{ct}", bufs=1)
                        for k4 in range(4):
                            kk = 4 * h + k4
                            for ct in range(2):
                                mm(pouts[(it, ct)][:],
                                   atf[h][:, k4, 128 * it:128 * it + 128],
                                   wout_sb[:, kk, 512 * ct:512 * ct + 512],
                                   start=(kk == 0), stop=(kk == KT - 1))
                        if h == 1:
                            for ct in range(2):
                                osb = p3s.tile([128, 512], F32, tag="osb",
                                               name="osb")
                                nc.vector.tensor_add(
                                    osb[:], pouts[(it, ct)][:],
                                    boutb[:, 512 * ct:512 * ct + 512])
                                eng = nc.sync if ct == 0 else nc.scalar
                                eng.dma_start(
                                    out_d[128 * it:128 * it + 128,
                                          512 * ct:512 * ct + 512], osb[:])


_NC_CACHE = {}

# test-only knobs: set TRACE=True before calling kernel() to profile; the
# BassKernelResults of the last run lands in LAST_RESULT.
TRACE = False
LAST_RESULT = None


def _get_nc():
    if "nc" not in _NC_CACHE:
        _NC_CACHE["nc"] = _build()
    return _NC_CACHE["nc"]


def _bf(a):
    return np.ascontiguousarray(a.astype(ml_dtypes.bfloat16))


def kernel(x, Wq, bq, Wkv, bkv, Wout, bout):
    x = np.asarray(x, np.float32)
    Wq = np.asarray(Wq, np.float32)
    bq = np.asarray(bq, np.float32)
    Wkv = np.asarray(Wkv, np.float32)
    bkv = np.asarray(bkv, np.float32)
    Wout = np.asarray(Wout, np.float32)
    bout = np.asarray(bout, np.float32)

    def ktile(a):  # [128*KT_rows, width] -> [128, KT_rows*width], row-linear
        kk = a.shape[0] // 128
        return np.ascontiguousarray(
            a.reshape(kk, 128, a.shape[1]).transpose(1, 0, 2).reshape(128, -1))

    def xtile(xtr):  # x^T [1024, 2048] -> [128, (quarter, ktile, 512)]
        return np.ascontiguousarray(
            xtr.reshape(KT, 128, NQ, 512).transpose(1, 2, 0, 3).reshape(128, -1))

    # additive causal mask for the diagonal tiles: -240*scale after the exp's
    # scale factor gives exp(-30) ~ 1e-13, i.e. an exact-enough zero
    mask = np.where(np.triu(np.ones((128, 128), np.float32)) > 0, 0.0,
                    -240.0).astype(np.float32)
    ident = np.eye(128, dtype=np.float32)
    xts = [xtile(np.ascontiguousarray(x[g].T)) for g in range(B)]
    # out-proj contraction row order: chunk kk = (pp, s) carries heads
    # (4*s + 2*pp, 4*s + 2*pp + 1) -> permute Wout rows to match
    wout_perm = np.concatenate(
        [Wout[256 * s + 128 * pp:256 * s + 128 * pp + 128]
         for pp in (0, 1) for s in range(4)])
    wout_t = ktile(wout_perm)
    in_maps = []
    for j in range(N_CORES):
        g, r = divmod(j, 4)
        cols = slice(HL * D * r, HL * D * (r + 1))
        in_maps.append({
            "xt": _bf(xts[g]),
            "wq": _bf(ktile(Wq[:, cols])),
            "wk": _bf(ktile(Wkv[:, 0:DIM][:, cols])),
            "wv": _bf(ktile(Wkv[:, DIM:2 * DIM][:, cols])),
            "wout": _bf(wout_t),
            "bq": np.ascontiguousarray(bq[cols][:, None]),
            "bk": np.ascontiguousarray(bkv[0:DIM][cols][:, None]),
            "bv": np.ascontiguousarray(bkv[DIM:2 * DIM][cols][None, :]),
            "bout": np.ascontiguousarray(bout[None, :]),
            "mask": _bf(mask),
            "ident": _bf(ident),
            "ones": _bf(np.ones((1, HL), np.float32)),
        })
    res = run_bass_kernel_spmd(_get_nc(), in_maps, list(range(N_CORES)),
                               trace=TRACE)
    global LAST_RESULT
    LAST_RESULT = res
    out = np.empty((B, N, DIM), np.float32)
    for j in range(N_CORES):
        g, r = divmod(j, 4)
        out[g, 512 * r:512 * (r + 1)] = res.results[j]["out"]
    return out


# revision 29
# speedup vs baseline: 1.0453x; 1.0453x over previous
"""Causal multi-head attention (b=2, n=2048, dim=1024, 16 heads) on 8 trn2
NeuronCores.

Sharding: core j = 4*g + r owns batch g and heads 4r..4r+3 (tensor parallel
over heads within each batch's 4-core group). Each core:
  P1  projects q/k (transposed layout [head_dim, tokens]) and v (natural
      [tokens, head_dim], ones-augmented) for its 4 heads from x^T, in bf16.
      xt is tiled by token-quarter so v/qk groups start as quarters land.
  P2  causal attention per head pair in S^T orientation: exp without max
      subtraction (scores are O(1)), triangular mask on diagonal tiles,
      O'^T = V_aug.T @ expS^T accumulated in PSUM (row 64 = softmax
      denominator Z), normalization via fast-approx reciprocal + DRAM-bounce
      partition broadcast.
  A2A one 4-core group AllToAll per head pair ([[0..3],[4..7]] replica
      groups, bf16, 512KB): transposes A^T from head-sharded to
      token-sharded. The first overlaps the second pair's attention.
  P3  out = A^T.T @ Wout for this core's 512-token block: the first
      contraction half runs during the second AllToAll, the rest after.
Host: transposes x per batch, slices weights per head group (bf16), gathers
the 8 [512, 1024] row blocks into the full [2, 2048, 1024] output.
"""
import numpy as np
import ml_dtypes

import concourse.bass as bass
import concourse.mybir as mybir
import concourse.tile as tile
from concourse.bass import AP, ds
from concourse.bass_utils import run_bass_kernel_spmd
from concourse.vector_clock import ScopedClock

F32 = mybir.dt.float32
BF16 = mybir.dt.bfloat16
EXP = mybir.ActivationFunctionType.Exp

N_CORES = 8
B, N, DIM, H = 2, 2048, 1024, 16
D = DIM // H                 # 64
HL = 4                       # heads per core
KT = DIM // 128              # 8 contraction k-tiles
NQ = 4                       # token quarters (512 tokens each)
NJ = N // 128                # 16 key tiles per batch
NI = N // 512                # 4 query i-blocks per batch
SCALE = float(D) ** -0.5
VS = 128                     # per-head stride in v tiles (bf16, 256B aligned)

MD = BF16


def _split_multi_waits(nc):
    """This walrus build rejects instructions carrying more than one sync
    wait. Hoist extra waits onto same-engine NoOps inserted directly before
    the offending instruction (engines execute their stream in order, so
    this preserves semantics)."""
    n = 0
    for f in nc.m.functions:
        for bb in f.blocks:
            insts = bb.instructions
            out = []
            changed = False
            for inst in insts:
                si = inst.sync_info
                waits = list(si.on_wait) if si is not None and si.on_wait else []
                if len(waits) > 1:
                    changed = True
                    for w in waits[:-1]:
                        nop = mybir.InstNoOp(name=f"I-waitfix-{n}", ins=[],
                                             outs=[])
                        n += 1
                        nop.engine = inst.engine
                        nop.sync_info = mybir.SyncInfo(on_wait=[w],
                                                       on_update=[])
                        out.append(nop)
                    si.on_wait = waits[-1:]
                out.append(inst)
            if changed:
                insts[:] = out
    return n


class _TC(tile.TileContext):
    """Tail drain in this walrus build only supports one sync-wait per CTRL
    instruction; spread the residual global-clock waits over SP nops, and
    split any remaining multi-wait instructions after scheduling."""

    def _drain_and_barrier(self, tick_clock, wait_clock):
        nop = self.nc.sync.nop()
        wait_clock.add_sem_waits(nop.ins, ScopedClock({None: tick_clock.global_clock}))
        si = nop.ins.sync_info
        waits = list(si.on_wait or []) if si is not None else []
        if len(waits) > 1:
            si.on_wait = waits[:1]
            for w in waits[1:]:
                extra = self.nc.sync.nop()
                extra.ins.sync_info = mybir.SyncInfo(on_wait=[w], on_update=[])
        self.nc.sync.drain()
        self.nc.all_engine_barrier()
        assert self.sems is not None
        popped = self.nc._tile_sem_poison_stack.pop()
        assert popped is self._sem_poison
        self.nc.clear_and_free_semaphores(list(self.sems.allocated().values()))
        self.nc.all_engine_barrier()

    def __exit__(self, exc_type, exc_val, exc_tb):
        r = super().__exit__(exc_type, exc_val, exc_tb)
        if exc_type is None:
            _split_multi_waits(self.nc)
        return r


def _bcast(src_dram_row, parts):
    """DRAM [1, n] row -> AP replicating it over `parts` partitions (step-0
    leading dim; only legal for DRAM sources)."""
    return AP(src_dram_row.tensor, src_dram_row.offset,
              [[0, parts]] + list(src_dram_row.ap)[1:])


def _build():
    nc = bass.Bass(trn_type="TRN2", target_bir_lowering=False, debug=False,
                   num_devices=N_CORES)
    dt = F32
    # xt pre-tiled on host: [128, (quarter, ktile, 512)] so each quarter is
    # one fully-linear 8KB-per-partition DMA and unlocks that quarter's
    # v/q/k projection groups.
    xt_d = nc.dram_tensor("xt", [128, NQ * KT * 512], MD, kind="ExternalInput").ap()
    wq_d = nc.dram_tensor("wq", [128, KT * HL * D], MD, kind="ExternalInput").ap()
    wk_d = nc.dram_tensor("wk", [128, KT * HL * D], MD, kind="ExternalInput").ap()
    wv_d = nc.dram_tensor("wv", [128, KT * HL * D], MD, kind="ExternalInput").ap()
    wout_d = nc.dram_tensor("wout", [128, KT * DIM], MD, kind="ExternalInput").ap()
    bq_d = nc.dram_tensor("bq", [HL * D, 1], dt, kind="ExternalInput").ap()
    bk_d = nc.dram_tensor("bk", [HL * D, 1], dt, kind="ExternalInput").ap()
    bv_d = nc.dram_tensor("bv", [1, HL * D], dt, kind="ExternalInput").ap()
    bout_d = nc.dram_tensor("bout", [1, DIM], dt, kind="ExternalInput").ap()
    mask_d = nc.dram_tensor("mask", [128, 128], MD, kind="ExternalInput").ap()
    ident_d = nc.dram_tensor("ident", [128, 128], MD, kind="ExternalInput").ap()
    ones_d = nc.dram_tensor("ones", [1, HL], MD, kind="ExternalInput").ap()
    out_d = nc.dram_tensor("out", [N // HL, DIM], dt, kind="ExternalOutput").ap()

    with _TC(nc) as tc, \
            nc.allow_low_precision(reason="bf16 matmul operand staging"):
        _body(nc, tc, xt_d, wq_d, wk_d, wv_d, wout_d, bq_d, bk_d, bv_d,
              bout_d, mask_d, ident_d, ones_d, out_d)
    return nc


def _body(nc, tc, xt_d, wq_d, wk_d, wv_d, wout_d, bq_d, bk_d, bv_d, bout_d,
          mask_d, ident_d, ones_d, out_d):
    mm = nc.tensor.matmul
    with tc.tile_pool(name="persist", bufs=1) as pers:
        # Persistent SBUF: q^T/k^T per head pair, v (ones-augmented) per
        # 128-token tile, mask, biases.
        qt = [pers.tile([128, N], MD, tag=f"qt{p}", name=f"qt{p}") for p in (0, 1)]
        kt = [pers.tile([128, N], MD, tag=f"kt{p}", name=f"kt{p}") for p in (0, 1)]
        vt = [pers.tile([128, HL * VS], MD, tag=f"v{t}", name=f"v{t}")
              for t in range(NJ)]
        mask_sb = pers.tile([128, 128], MD, tag="mask", name="mask_sb")
        ident_sb = pers.tile([128, 128], MD, tag="ident", name="ident_sb")
        bqc = pers.tile([128, 2], F32, tag="bqc", name="bqc")
        bkc = pers.tile([128, 2], F32, tag="bkc", name="bkc")
        bvb = pers.tile([128, HL * D], F32, tag="bvb", name="bvb")
        boutb = pers.tile([128, DIM], F32, tag="boutb", name="boutb")
        ones_sb = pers.tile([128, HL], MD, tag="ones", name="ones_sb")

        # ---------------- P1: projections ----------------
        with (tc.tile_pool(name="p1s", bufs=1) as p1s,
              tc.tile_pool(name="pqk", bufs=4, space="PSUM") as pqk,
              tc.tile_pool(name="pv", bufs=3, space="PSUM") as pvp):
            w_sb = {}
            for nm in ("wq", "wk", "wv"):
                w_sb[nm] = p1s.tile([128, KT, HL * D], MD, tag=nm, name=f"{nm}_sb")
            xt_sb = p1s.tile([128, NQ, KT, 512], MD, tag="xt", name="xt_sb")
            gate = p1s.tile([1, 4], F32, tag="gate", name="gate")
            xt_dv = xt_d.rearrange("p (q k n) -> p q k n", q=NQ, k=KT)
            # first-needed tensors ride the empty HWDGE queues; the rest go
            # via gpsimd SWDGE. Quarters 1-3 are gated behind a consumer of
            # the previous quarter (the SDMA engines round-robin all queued
            # work, so ungated issue would starve quarter 0).
            nc.sync.dma_start(w_sb["wq"][:],
                              wq_d.rearrange("p (k e) -> p k e", k=KT))
            nc.scalar.dma_start(w_sb["wk"][:],
                                wk_d.rearrange("p (k e) -> p k e", k=KT))
            nc.gpsimd.dma_start(w_sb["wv"][:],
                                wv_d.rearrange("p (k e) -> p k e", k=KT))
            nc.sync.dma_start(xt_sb[:, 0, :, :], xt_dv[:, 0, :, :])
            nc.gpsimd.dma_start(mask_sb[:], mask_d[:])
            nc.gpsimd.dma_start(ident_sb[:], ident_d[:])
            nc.gpsimd.dma_start(bqc[:],
                                bq_d.rearrange("(m p) o -> p (m o)", p=128))
            nc.gpsimd.dma_start(bkc[:],
                                bk_d.rearrange("(m p) o -> p (m o)", p=128))
            nc.gpsimd.dma_start(bvb[:], _bcast(bv_d[0:1, :], 128))
            nc.gpsimd.dma_start(boutb[:], _bcast(bout_d[0:1, :], 128))
            nc.gpsimd.dma_start(ones_sb[:], _bcast(ones_d[0:1, :], 128))
            # warm the exp table during the input DMA wait (first real exp
            # otherwise pays the ~2.7us ACT table load inside P2)
            nc.scalar.activation(gate[0:1, 0:1], bqc[0:1, 0:1], EXP, scale=0.0)

            for ntq in range(NQ):
                if ntq > 0:
                    # gate quarter ntq's load behind first use of the
                    # previous quarter (vt tile of its first token tile)
                    nc.gpsimd.tensor_add(gate[0:1, ntq:ntq + 1],
                                         xt_sb[0:1, ntq, 0, 0:1],
                                         vt[4 * (ntq - 1)][0:1, 0:1])
                    eng = (nc.scalar, nc.sync, nc.gpsimd)[ntq - 1]
                    eng.dma_start(xt_sb[:, ntq, :, :], xt_dv[:, ntq, :, :])
                # v projections for this quarter's 4 token tiles
                for tl in range(4):
                    tt = 4 * ntq + tl
                    ps = pvp.tile([128, HL * D], F32, tag="pv", name="ps_v")
                    for kk in range(KT):
                        mm(ps[:],
                           xt_sb[:, ntq, kk, 128 * tl:128 * tl + 128],
                           w_sb["wv"][:, kk, :],
                           start=(kk == 0), stop=(kk == KT - 1))
                    vv = vt[tt].rearrange("p (h x) -> p h x", x=VS)
                    nc.vector.tensor_add(vv[:, :, 0:D],
                                         ps.rearrange("p (h x) -> p h x", x=D),
                                         bvb.rearrange("p (h x) -> p h x", x=D))
                    nc.vector.tensor_copy(
                        vv[:, :, D:D + 1],
                        ones_sb.rearrange("p (h o) -> p h o", o=1))
                # q/k projections for this quarter's 512 tokens
                for w, bcol, dst in (("wq", bqc, qt), ("wk", bkc, kt)):
                    for mt in (0, 1):
                        ps = pqk.tile([128, 512], F32, tag="pqk", name="ps_qk")
                        for kk in range(KT):
                            mm(ps[:],
                               w_sb[w][:, kk, 128 * mt:128 * mt + 128],
                               xt_sb[:, ntq, kk, :],
                               start=(kk == 0), stop=(kk == KT - 1))
                        nc.vector.tensor_scalar_add(
                            dst[mt][:, 512 * ntq:512 * ntq + 512], ps[:],
                            bcol[:, mt:mt + 1])

        # wout prefetch: gated behind the end of P1 so it does not steal
        # HBM bandwidth from the xt quarters; it only needs to land before
        # P3-h0 (~60us later).
        with tc.tile_pool(name="p3w", bufs=1) as p3w:
            wout_sb = p3w.tile([128, KT, DIM], MD, tag="wout", name="wout_sb")
            wgate = p3w.tile([1, 1], F32, tag="wgate", name="wgate")
            nc.gpsimd.tensor_add(wgate[0:1, 0:1], wout_sb[0:1, 0, 0:1],
                                 qt[1][0:1, N - 1:N])
            nc.gpsimd.dma_start(wout_sb[:],
                                wout_d.rearrange("p (k c) -> p k c", k=KT))
            atf = [p3w.tile([128, 4, 512], MD, tag=f"atf{h}", name=f"atf{h}")
                   for h in (0, 1)]

            # ---------------- P2: attention + streamed a2a ----------------
            pid = nc.sync.partition_id()
            gsel = nc.sync.snap(pid // 4, min_val=0, max_val=1)
            with (tc.tile_pool(name="p2s", bufs=4) as p2s,
                  tc.tile_pool(name="p2n", bufs=2) as p2n,
                  tc.tile_pool(name="dram", bufs=1, space="DRAM") as dram,
                  tc.tile_pool(name="p2d", bufs=2, space="DRAM") as p2d,
                  tc.tile_pool(name="sp", bufs=2, space="PSUM") as sp,
                  tc.tile_pool(name="op", bufs=2, space="PSUM") as op):
                # one 8-core AllToAll per head pair (bf16 halves the wire
                # bytes) so the first overlaps the second pair's attention.
                # Every core writes its chunks at BOTH groups' positions
                # (static offsets); receivers pick their group's half with
                # one dynamic read.
                a2a_in = [dram.tile([1024, 512], MD, name=f"a2a_in{h}")
                          for h in (0, 1)]
                a2a_out = [dram.tile([1024, 512], MD, name=f"a2a_out{h}")
                           for h in (0, 1)]
                for pp in (0, 1):
                    for I in range(NI):
                        i0 = 512 * I
                        last = 4 * I + 3
                        poA = op.tile([D + 1, 512], F32, tag="oA", name="poA")
                        poB = op.tile([D + 1, 512], F32, tag="oB", name="poB")
                        for jj in range(4 * I + 4):
                            di = jj - 4 * I
                            f0 = 128 * di if di >= 0 else 0
                            diag = di >= 0
                            ps = sp.tile([128, 1024], F32, tag="s", name="ps_s")
                            mm(ps[:, f0:512],
                               kt[pp][0:64, 128 * jj:128 * jj + 128],
                               qt[pp][0:64, i0 + f0:i0 + 512],
                               start=True, stop=not diag)
                            mm(ps[:, 512 + f0:1024],
                               kt[pp][64:128, 128 * jj:128 * jj + 128],
                               qt[pp][64:128, i0 + f0:i0 + 512],
                               start=True, stop=not diag)
                            if diag:
                                # additive causal mask accumulated on the PE
                                # (I.T @ maskneg = maskneg): keeps the mask
                                # off the DVE/gpsimd streams entirely
                                mm(ps[:, f0:f0 + 128], ident_sb[:],
                                   mask_sb[:], start=False, stop=True,
                                   skip_group_check=True)
                                mm(ps[:, 512 + f0:512 + f0 + 128],
                                   ident_sb[:], mask_sb[:],
                                   start=False, stop=True,
                                   skip_group_check=True)
                            e = p2s.tile([128, 1024], MD, tag="e", name="e_s")
                            ev = e.rearrange("p (h x) -> p h x", x=512)
                            pv2 = ps.rearrange("p (h x) -> p h x", x=512)
                            nc.scalar.activation(ev[:, :, f0:512],
                                                 pv2[:, :, f0:512], EXP,
                                                 scale=SCALE)
                            vv = vt[jj].rearrange("p (h x) -> p h x", x=VS)
                            mm(poA[:, f0:512], vv[:, 2 * pp, 0:D + 1],
                               e[:, f0:512],
                               start=(jj == 0), stop=(jj == last))
                            mm(poB[:, f0:512], vv[:, 2 * pp + 1, 0:D + 1],
                               e[:, 512 + f0:1024],
                               start=(jj == 0), stop=(jj == last))
                        # normalization: Z sits in row 64 of each O' psum.
                        # 1/Z (fast approx) on partition 64, bounced via DRAM
                        # to broadcast across partitions (step-0 DMA).
                        tail_blk = pp == 1 and I == NI - 1
                        oc = p2n.tile([128, 1024], F32, tag="oc", name="oc")
                        # tail block: the whole normalize chain runs on
                        # ACT (idle after the last exp) + gpsimd (idle after
                        # the first collective) so it never queues behind
                        # the vector engine's in-order backlog
                        if tail_blk:
                            nc.scalar.copy(oc[0:65, 0:512], poA[0:65, :])
                            nc.scalar.copy(oc[0:65, 512:1024], poB[0:65, :])
                        else:
                            nc.vector.tensor_copy(oc[0:65, 0:512],
                                                  poA[0:65, :])
                            nc.vector.tensor_copy(oc[0:65, 512:1024],
                                                  poB[0:65, :])
                        zrow = p2n.tile([1, 1024], F32, tag="zrow", name="zrow")
                        zdram = p2d.tile([1, 1024], F32, tag="zdram", name="zdram")
                        if tail_blk:
                            # the ACT engine is idle after the last exp and
                            # 1/Z = exp(-ln Z) there is ~3x faster than the
                            # DVE reciprocal that would otherwise sit on the
                            # critical tail chain
                            lnz = p2n.tile([1, 1024], F32, tag="lnz",
                                           name="lnz")
                            nc.scalar.activation(
                                lnz[0:1, :], oc[64:65, :],
                                mybir.ActivationFunctionType.Ln)
                            nc.scalar.activation(zrow[0:1, :], lnz[0:1, :],
                                                 EXP, scale=-1.0)
                            nc.sync.dma_start(zdram[0:1, :], zrow[0:1, :])
                        else:
                            # split per half so the DRAM bounce of half A
                            # overlaps the reciprocal of half B
                            nc.vector.reciprocal(zrow[0:1, 0:512],
                                                 oc[64:65, 0:512])
                            nc.sync.dma_start(zdram[0:1, 0:512],
                                              zrow[0:1, 0:512])
                            nc.vector.reciprocal(zrow[0:1, 512:1024],
                                                 oc[64:65, 512:1024])
                            nc.sync.dma_start(zdram[0:1, 512:1024],
                                              zrow[0:1, 512:1024])
                        rzb = p2n.tile([64, 1024], F32, tag="rzb", name="rzb")
                        nc.sync.dma_start(rzb[:, 0:512],
                                          _bcast(zdram[0:1, 0:512], 64))
                        nc.sync.dma_start(rzb[:, 512:1024],
                                          _bcast(zdram[0:1, 512:1024], 64))
                        stA = p2n.tile([64, 512], MD, tag="stA", name="stA")
                        stB = p2n.tile([64, 512], MD, tag="stB", name="stB")
                        meng = nc.gpsimd if tail_blk else nc.vector
                        meng.tensor_mul(stA[:], oc[0:64, 0:512],
                                        rzb[:, 0:512])
                        meng.tensor_mul(stB[:], oc[0:64, 512:1024],
                                        rzb[:, 512:1024])
                        for gg in (0, 1):
                            r0 = 512 * gg + 128 * I
                            nc.sync.dma_start(a2a_in[pp][r0:r0 + 64, :], stA[:])
                            nc.sync.dma_start(a2a_in[pp][r0 + 64:r0 + 128, :],
                                              stB[:])
                    nc.gpsimd.collective_compute(
                        "AllToAll", mybir.AluOpType.bypass,
                        replica_groups=[list(range(N_CORES))],
                        ins=[a2a_in[pp].opt()], outs=[a2a_out[pp].opt()])
                    av = a2a_out[pp].rearrange("(G s p) c -> p G s c",
                                               s=4, p=128)
                    for s in range(4):
                        nc.sync.dma_start(atf[pp][:, s, :],
                                          av[:, ds(gsel, 1), s, :])

            # ---------------- P3: output projection ----------------
            # first contraction half (head pair 0) runs while the second
            # AllToAll is still in flight; second half lands on top.
            # Both ct halves of a (it, k4) pair share the stationary operand.
            with (tc.tile_pool(name="p3s", bufs=2) as p3s,
                  tc.tile_pool(name="p3p", bufs=8, space="PSUM") as p3p):
                pouts = {}
                for h in (0, 1):
                    for it in range(4):
                        for ct in range(2):
                            if h == 0:
                                pouts[(it, ct)] = p3p.tile(
                                    [128, 512], F32, tag=f"po{it}{ct}",
                                    name=f"po{it}{ct}", bufs=1)
                        for k4 in range(4):
                            kk = 4 * h + k4
                            for ct in range(2):
                                mm(pouts[(it, ct)][:],
                                   atf[h][:, k4, 128 * it:128 * it + 128],
                                   wout_sb[:, kk, 512 * ct:512 * ct + 512],
                                   start=(kk == 0), stop=(kk == KT - 1))
                        if h == 1:
                            for ct in range(2):
                                osb = p3s.tile([128, 512], F32, tag="osb",
                                               name="osb")
                                nc.vector.tensor_add(
                                    osb[:], pouts[(it, ct)][:],
                                    boutb[:, 512 * ct:512 * ct + 512])
                                eng = nc.sync if ct == 0 else nc.scalar
                                eng.dma_start(
                                    out_d[128 * it:128 * it + 128,
                                          512 * ct:512 * ct + 512], osb[:])


_NC_CACHE = {}

# test-only knobs: set TRACE=True before calling kernel() to profile; the
# BassKernelResults of the last run lands in LAST_RESULT.
TRACE = False
LAST_RESULT = None


def _get_nc():
    if "nc" not in _NC_CACHE:
        _NC_CACHE["nc"] = _build()
    return _NC_CACHE["nc"]


def _bf(a):
    return np.ascontiguousarray(a.astype(ml_dtypes.bfloat16))


def kernel(x, Wq, bq, Wkv, bkv, Wout, bout):
    x = np.asarray(x, np.float32)
    Wq = np.asarray(Wq, np.float32)
    bq = np.asarray(bq, np.float32)
    Wkv = np.asarray(Wkv, np.float32)
    bkv = np.asarray(bkv, np.float32)
    Wout = np.asarray(Wout, np.float32)
    bout = np.asarray(bout, np.float32)

    def ktile(a):  # [128*KT_rows, width] -> [128, KT_rows*width], row-linear
        kk = a.shape[0] // 128
        return np.ascontiguousarray(
            a.reshape(kk, 128, a.shape[1]).transpose(1, 0, 2).reshape(128, -1))

    def xtile(xtr):  # x^T [1024, 2048] -> [128, (quarter, ktile, 512)]
        return np.ascontiguousarray(
            xtr.reshape(KT, 128, NQ, 512).transpose(1, 2, 0, 3).reshape(128, -1))

    # additive causal mask for the diagonal tiles: -240*scale after the exp's
    # scale factor gives exp(-30) ~ 1e-13, i.e. an exact-enough zero
    mask = np.where(np.triu(np.ones((128, 128), np.float32)) > 0, 0.0,
                    -240.0).astype(np.float32)
    ident = np.eye(128, dtype=np.float32)
    xts = [xtile(np.ascontiguousarray(x[g].T)) for g in range(B)]
    # out-proj contraction row order: chunk kk = (pp, s) carries heads
    # (4*s + 2*pp, 4*s + 2*pp + 1) -> permute Wout rows to match
    wout_perm = np.concatenate(
        [Wout[256 * s + 128 * pp:256 * s + 128 * pp + 128]
         for pp in (0, 1) for s in range(4)])
    wout_t = ktile(wout_perm)
    in_maps = []
    for j in range(N_CORES):
        g, r = divmod(j, 4)
        cols = slice(HL * D * r, HL * D * (r + 1))
        in_maps.append({
            "xt": _bf(xts[g]),
            "wq": _bf(ktile(Wq[:, cols])),
            "wk": _bf(ktile(Wkv[:, 0:DIM][:, cols])),
            "wv": _bf(ktile(Wkv[:, DIM:2 * DIM][:, cols])),
            "wout": _bf(wout_t),
            "bq": np.ascontiguousarray(bq[cols][:, None]),
            "bk": np.ascontiguousarray(bkv[0:DIM][cols][:, None]),
            "bv": np.ascontiguousarray(bkv[DIM:2 * DIM][cols][None, :]),
            "bout": np.ascontiguousarray(bout[None, :]),
            "mask": _bf(mask),
            "ident": _bf(ident),
            "ones": _bf(np.ones((1, HL), np.float32)),
        })
    res = run_bass_kernel_spmd(_get_nc(), in_maps, list(range(N_CORES)),
                               trace=TRACE)
    global LAST_RESULT
    LAST_RESULT = res
    out = np.empty((B, N, DIM), np.float32)
    for j in range(N_CORES):
        g, r = divmod(j, 4)
        out[g, 512 * r:512 * (r + 1)] = res.results[j]["out"]
    return out
